# revision 1
# baseline (speedup 1.0000x reference)
"""Chunkwise causal attention (B=2, S=4096, H=16, D=64, CHUNK=128) on 8 TRN2 NeuronCores.

Sharding: head-parallel tensor parallelism. Core c owns heads (2c, 2c+1) for both
batches: it computes the qkv projection for its heads (w_qkv column slice), full
causal attention for its 4 (batch, head) units, and a partial out-projection
(w_out row slice). Host sums the 8 partial outputs.

Device kernel layout notes:
 - x is passed host-transposed as xT [1024, 8192] bf16 so the qkv contraction dim
   (hidden) lands on SBUF partitions without any device-side transpose.
 - Q^T, K^T are kept head-major [128 = 2 heads x 64, S]; V is kept key-major
   [keys, 2, 65] with a ones column so the P@V matmul also produces the softmax
   denominator (row 64 of the PV psum).
 - Scores are computed transposed (scores^T [keys, queries]) so the exp'd
   probabilities are directly the moving operand of the P@V matmul - no
   transposes anywhere on the device.
 - Softmax skips max-subtraction (scores ~ N(0,1): exp never overflows in f32);
   causal masking multiplies the exp'd diagonal blocks by a precomputed 0/1 mask.
"""

import sys

if "/opt/trn_rl_repo" not in sys.path:
    sys.path.insert(0, "/opt/trn_rl_repo")

import numpy as np
import ml_dtypes

B = 2
S = 4096
HID = 1024
NHEAD = 16
D = 64
CH = 128  # key chunk (= reference CHUNK)
G = 512  # query group (4 chunks)
NGB = S // G  # 8 query groups per batch
KK = HID // 128  # 8 contraction chunks for the projections
NKC = S // CH  # 32 key chunks per batch
TT = B * S  # 8192 tokens across batches

_CACHE = {}


def _build_nc(reps=1):
    import concourse.mybir as mybir
    import concourse.tile as tile
    from concourse import bacc
    from contextlib import ExitStack

    bf16 = mybir.dt.bfloat16
    f32 = mybir.dt.float32
    Exp = mybir.ActivationFunctionType.Exp
    mult = mybir.AluOpType.mult

    nc = bacc.Bacc("TRN2", target_bir_lowering=False, debug=False)
    xT_d = nc.dram_tensor("xT", [HID, TT], bf16, kind="ExternalInput")
    wq_d = nc.dram_tensor("wq", [HID, 128], bf16, kind="ExternalInput")
    wk_d = nc.dram_tensor("wk", [HID, 128], bf16, kind="ExternalInput")
    wv_d = nc.dram_tensor("wv", [HID, 128], bf16, kind="ExternalInput")
    wo_d = nc.dram_tensor("wo", [128, HID], bf16, kind="ExternalInput")
    mask_d = nc.dram_tensor("mask", [128, G + 6 * CH], bf16, kind="ExternalInput")
    out_d = nc.dram_tensor("out", [TT, HID], f32, kind="ExternalOutput")

    xT_r = xT_d.rearrange("(kk p) t -> p kk t", p=128)
    wq_r = wq_d.rearrange("(kk p) c -> p kk c", p=128)
    wk_r = wk_d.rearrange("(kk p) c -> p kk c", p=128)
    wv_r = wv_d.rearrange("(kk p) c -> p kk c", p=128)

    with tile.TileContext(nc) as tc, ExitStack() as ctx:
        consts = ctx.enter_context(tc.tile_pool(name="consts", bufs=1))
        qkv_pool = ctx.enter_context(tc.tile_pool(name="qkv", bufs=2))
        xt_pool = ctx.enter_context(tc.tile_pool(name="xt", bufs=4))
        exp_pool = ctx.enter_context(tc.tile_pool(name="exp", bufs=6))
        attn_pool = ctx.enter_context(tc.tile_pool(name="attn", bufs=6))
        norm_pool = ctx.enter_context(tc.tile_pool(name="norm", bufs=6))
        osb_pool = ctx.enter_context(tc.tile_pool(name="osb", bufs=3))
        ps_mm = ctx.enter_context(tc.tile_pool(name="psmm", bufs=2, space="PSUM"))
        ps_sq = ctx.enter_context(tc.tile_pool(name="pssq", bufs=2, space="PSUM"))
        ps_pv = ctx.enter_context(tc.tile_pool(name="pspv", bufs=2, space="PSUM"))

        wq_sb = consts.tile([128, KK, 128], bf16, tag="wq")
        wk_sb = consts.tile([128, KK, 128], bf16, tag="wk")
        wv_sb = consts.tile([128, KK, 128], bf16, tag="wv")
        wo_sb = consts.tile([128, HID], bf16, tag="wo")
        mask_sb = consts.tile([128, G + 6 * CH], bf16, tag="mask")
        ones_sb = consts.tile([1, 64], bf16, tag="ones")
        nc.sync.dma_start(wq_sb[:], wq_r)
        nc.sync.dma_start(wk_sb[:], wk_r)
        nc.sync.dma_start(wv_sb[:], wv_r)
        nc.sync.dma_start(wo_sb[:], wo_d[:])
        nc.sync.dma_start(mask_sb[:], mask_d[:])
        nc.vector.memset(ones_sb[:], 1.0)

        def finish_group(pvs, t0):
            # normalize (unnormalized PV x broadcast reciprocal) + out-projection
            rec = norm_pool.tile([1, 2 * G], bf16, tag="rec")
            with nc.allow_low_precision(reason="softmax denominator reciprocal in bf16"):
                nc.vector.reciprocal(rec[:], pvs[64:65, :])
            bcp = ps_mm.tile([128, G], f32, tag="mm")
            nc.tensor.matmul(bcp[0:64, :], ones_sb[:], rec[0:1, 0:G])
            nc.tensor.matmul(
                bcp[64:128, :], ones_sb[:], rec[0:1, G : 2 * G], tile_position=(0, 64)
            )
            at = attn_pool.tile([128, G], bf16, tag="attnT")
            nc.vector.tensor_tensor(at[0:64, :], pvs[0:64, 0:G], bcp[0:64, :], op=mult)
            nc.vector.tensor_tensor(
                at[64:128, :], pvs[0:64, G : 2 * G], bcp[64:128, :], op=mult
            )
            for tch in range(G // CH):
                for nn in range(2):
                    pso = ps_mm.tile([128, G], f32, tag="mm")
                    nc.tensor.matmul(
                        pso[:],
                        at[:, tch * CH : (tch + 1) * CH],
                        wo_sb[:, nn * G : (nn + 1) * G],
                    )
                    ob = osb_pool.tile([128, G], f32, tag="ob")
                    nc.vector.tensor_copy(ob[:], pso[:])
                    nc.sync.dma_start(
                        out_d[t0 + tch * CH : t0 + (tch + 1) * CH, nn * G : (nn + 1) * G],
                        ob[:],
                    )

        pending = None
        for _rep in range(reps):
            QTb, KTb, Vb = [], [], []
            for b in range(B):
                QTb.append(qkv_pool.tile([128, S], bf16, tag=f"QT{b}", name=f"QT{b}"))
                KTb.append(qkv_pool.tile([128, S], bf16, tag=f"KT{b}", name=f"KT{b}"))
                Vb.append(qkv_pool.tile([128, NKC, 2, 65], bf16, tag=f"V{b}", name=f"V{b}"))
                nc.gpsimd.memset(Vb[b][:, :, :, 64:65], 1.0)

            # interleave the two batches' query groups: (b0,g0),(b1,g0),(b0,g1),...
            # so small-g groups never drain the PE/ACT pipeline.
            for g, b in [(gg, bb) for gg in range(NGB) for bb in range(B)]:
                QT, KT, V = QTb[b], KTb[b], Vb[b]
                t0 = b * S + g * G

                # ---- phase 1: qkv projection for this token group ----
                xt = xt_pool.tile([128, KK, G], bf16, tag="xt")
                nc.sync.dma_start(xt[:], xT_r[:, :, t0 : t0 + G])
                for w_sb, dstT in ((wq_sb, QT), (wk_sb, KT)):
                    ps = ps_mm.tile([128, G], f32, tag="mm")
                    for kk in range(KK):
                        nc.tensor.matmul(
                            ps[:],
                            w_sb[:, kk, :],
                            xt[:, kk, :],
                            start=(kk == 0),
                            stop=(kk == KK - 1),
                        )
                    nc.vector.tensor_copy(dstT[:, g * G : (g + 1) * G], ps[:])
                for tch in range(G // CH):
                    ps = ps_mm.tile([128, G], f32, tag="mm")
                    for kk in range(KK):
                        nc.tensor.matmul(
                            ps[:, 0:CH],
                            xt[:, kk, tch * CH : (tch + 1) * CH],
                            wv_sb[:, kk, :],
                            start=(kk == 0),
                            stop=(kk == KK - 1),
                        )
                    kc = g * 4 + tch
                    nc.vector.tensor_copy(
                        V[:, kc, :, 0:64],
                        ps[:, 0:CH].rearrange("p (h d) -> p h d", h=2),
                    )

                # ---- phase 2: attention for query group g (keys 0..4g+3) ----
                nkc = 4 * g + 4  # causal key chunks for this group
                pv = [ps_pv.tile([65, G], f32, tag="pv", name=f"pv{h}") for h in range(2)]
                # packed mask offsets: jg=0..3 regions at [0:512],[512:896],[896:1152],[1152:1280]
                MOFF = [0, G, G + 3 * CH, G + 3 * CH + 2 * CH]
                for qd in range(nkc // 2):
                    # per-block trim: diagonal-band chunk jg only attends q >= jg*128,
                    # so its scores/exp/PV only cover N = 512 - jg*128 columns.
                    kcs, qoffs, Ns, offs = [], [], [], []
                    off = 0
                    for j in range(2):
                        kc = qd * 2 + j
                        jg = kc - (nkc - 4)
                        qoff = max(jg, 0) * CH
                        kcs.append(kc)
                        qoffs.append(qoff)
                        Ns.append(G - qoff)
                        offs.append(off)
                        off += G - qoff
                    sq = [ps_sq.tile([128, 2 * G], f32, tag="sq", name=f"sq{h}") for h in range(2)]
                    for j in range(2):
                        for h in range(2):
                            nc.tensor.matmul(
                                sq[h][:, offs[j] : offs[j] + Ns[j]],
                                KT[h * 64 : (h + 1) * 64, kcs[j] * CH : (kcs[j] + 1) * CH],
                                QT[h * 64 : (h + 1) * 64, g * G + qoffs[j] : (g + 1) * G],
                            )
                    ex = [exp_pool.tile([128, 2 * G], bf16, tag="exp", name=f"ex{h}") for h in range(2)]
                    for h in range(2):
                        # exp(scores / sqrt(D)); scale folded into the ACT affine
                        nc.scalar.activation(
                            ex[h][:, 0:off], sq[h][:, 0:off], Exp, scale=0.125
                        )
                    if qd >= nkc // 2 - 2:
                        seg = qd - (nkc // 2 - 2)  # 0 -> jg 0/1 region, 1 -> jg 2/3
                        moff = MOFF[2 * seg]
                        for h in range(2):
                            nc.vector.tensor_tensor(
                                ex[h][:, 0:off],
                                ex[h][:, 0:off],
                                mask_sb[:, moff : moff + off],
                                op=mult,
                            )
                    for j in range(2):
                        for h in range(2):
                            nc.tensor.matmul(
                                pv[h][:, qoffs[j] : G],
                                V[:, kcs[j], h, :],
                                ex[h][:, offs[j] : offs[j] + Ns[j]],
                                start=(kcs[j] == 0),
                                stop=(kcs[j] == nkc - 1),
                            )

                # free the pv psum banks immediately; normalize+outproj for this
                # group are emitted AFTER the next group's matmuls (1-group lag)
                # so the PE program never blocks on the DVE normalize chain.
                pvs = norm_pool.tile([65, 2 * G], f32, tag="pvs")
                nc.vector.tensor_copy(pvs[:, 0:G], pv[0][:])
                nc.vector.tensor_copy(pvs[:, G : 2 * G], pv[1][:])
                if pending is not None:
                    finish_group(*pending)
                pending = (pvs, t0)
        if pending is not None:
            finish_group(*pending)
    nc.compile()
    return nc


def _causal_mask():
    # Packed per-region triangular mask. Every diagonal-band chunk jg reduces to
    # the same pattern tri[k, qq] = (qq >= k) truncated to width 512-128*jg;
    # regions are packed back to back: widths 512, 384, 256, 128.
    tri = (np.arange(G)[None, :] >= np.arange(128)[:, None]).astype(np.float32)
    parts = [tri[:, 0 : G - jg * CH] for jg in range(4)]
    return np.concatenate(parts, axis=1).astype(ml_dtypes.bfloat16)


def get_nc(reps=1):
    key = f"nc{reps}"
    if key not in _CACHE:
        _CACHE[key] = _build_nc(reps)
    return _CACHE[key]


def make_in_maps(x, w_qkv, w_out):
    bf16 = ml_dtypes.bfloat16
    xf = np.asarray(x, dtype=np.float32).reshape(TT, HID)
    xT = np.ascontiguousarray(xf.T).astype(bf16)
    wqkv = np.asarray(w_qkv, dtype=np.float32)
    wout = np.asarray(w_out, dtype=np.float32)
    mask = _causal_mask()
    in_maps = []
    for c in range(8):
        c0 = 128 * c
        in_maps.append(
            {
                "xT": xT,
                "wq": np.ascontiguousarray(wqkv[:, c0 : c0 + 128]).astype(bf16),
                "wk": np.ascontiguousarray(wqkv[:, HID + c0 : HID + c0 + 128]).astype(bf16),
                "wv": np.ascontiguousarray(wqkv[:, 2 * HID + c0 : 2 * HID + c0 + 128]).astype(bf16),
                "wo": np.ascontiguousarray(wout[c0 : c0 + 128, :]).astype(bf16),
                "mask": mask,
            }
        )
    return in_maps


def kernel(x, w_qkv, w_out):
    from concourse.bass_utils import run_bass_kernel_spmd

    nc = get_nc()
    in_maps = make_in_maps(x, w_qkv, w_out)
    res = run_bass_kernel_spmd(nc, in_maps, core_ids=list(range(8)))
    acc = np.zeros((TT, HID), dtype=np.float32)
    for r in res.results:
        acc += r["out"]
    return acc.reshape(B, S, HID)



# revision 9
# speedup vs baseline: 300.0297x; 300.0297x over previous
"""Chunkwise causal attention (B=2, S=4096, H=16, D=64, CHUNK=128) on 8 TRN2 NeuronCores.

Sharding: head-parallel tensor parallelism. Core c owns heads (2c, 2c+1) for both
batches: it computes the qkv projection for its heads (w_qkv column slice), full
causal attention for its 4 (batch, head) units, and a partial out-projection
(w_out row slice). Host sums the 8 partial outputs.

Device kernel layout notes:
 - x is passed host-transposed as xT [1024, 8192] bf16 so the qkv contraction dim
   (hidden) lands on SBUF partitions without any device-side transpose.
 - Q^T, K^T are kept head-major [128 = 2 heads x 64, S]; V is kept key-major
   [keys, 2, 65] with a ones column so the P@V matmul also produces the softmax
   denominator (row 64 of the PV psum).
 - Scores are computed transposed (scores^T [keys, queries]) so the exp'd
   probabilities are directly the moving operand of the P@V matmul - no
   transposes anywhere on the device.
 - Softmax skips max-subtraction (scores ~ N(0,1): exp never overflows in f32);
   causal masking multiplies the exp'd diagonal blocks by a precomputed 0/1 mask.
"""

import sys

if "/opt/trn_rl_repo" not in sys.path:
    sys.path.insert(0, "/opt/trn_rl_repo")

import numpy as np
import ml_dtypes

B = 2
S = 4096
HID = 1024
NHEAD = 16
D = 64
CH = 128  # key chunk (= reference CHUNK)
G = 512  # query group (4 chunks)
NGB = S // G  # 8 query groups per batch
KK = HID // 128  # 8 contraction chunks for the projections
NKC = S // CH  # 32 key chunks per batch
TT = B * S  # 8192 tokens across batches

_CACHE = {}


def _build_nc(reps=1):
    import concourse.mybir as mybir
    import concourse.tile as tile
    from concourse import bacc
    from contextlib import ExitStack

    bf16 = mybir.dt.bfloat16
    f32 = mybir.dt.float32
    Exp = mybir.ActivationFunctionType.Exp
    mult = mybir.AluOpType.mult

    nc = bacc.Bacc("TRN2", target_bir_lowering=False, debug=False)
    xT_d = nc.dram_tensor("xT", [HID, TT], bf16, kind="ExternalInput")
    wq_d = nc.dram_tensor("wq", [HID, 128], bf16, kind="ExternalInput")
    wk_d = nc.dram_tensor("wk", [HID, 128], bf16, kind="ExternalInput")
    wv_d = nc.dram_tensor("wv", [HID, 128], bf16, kind="ExternalInput")
    wo_d = nc.dram_tensor("wo", [128, HID], bf16, kind="ExternalInput")
    mask_d = nc.dram_tensor("mask", [128, G + 6 * CH], bf16, kind="ExternalInput")
    out_d = nc.dram_tensor("out", [TT, HID], f32, kind="ExternalOutput")

    xT_r = xT_d.rearrange("(kk p) t -> p kk t", p=128)
    wq_r = wq_d.rearrange("(kk p) c -> p kk c", p=128)
    wk_r = wk_d.rearrange("(kk p) c -> p kk c", p=128)
    wv_r = wv_d.rearrange("(kk p) c -> p kk c", p=128)

    with tile.TileContext(nc) as tc, ExitStack() as ctx:
        consts = ctx.enter_context(tc.tile_pool(name="consts", bufs=1))
        qkv_pool = ctx.enter_context(tc.tile_pool(name="qkv", bufs=2))
        xt_pool = ctx.enter_context(tc.tile_pool(name="xt", bufs=4))
        exp_pool = ctx.enter_context(tc.tile_pool(name="exp", bufs=6))
        attn_pool = ctx.enter_context(tc.tile_pool(name="attn", bufs=6))
        norm_pool = ctx.enter_context(tc.tile_pool(name="norm", bufs=6))
        osb_pool = ctx.enter_context(tc.tile_pool(name="osb", bufs=3))
        ps_mm = ctx.enter_context(tc.tile_pool(name="psmm", bufs=2, space="PSUM"))
        ps_sq = ctx.enter_context(tc.tile_pool(name="pssq", bufs=2, space="PSUM"))
        ps_pv = ctx.enter_context(tc.tile_pool(name="pspv", bufs=2, space="PSUM"))

        wq_sb = consts.tile([128, KK, 128], bf16, tag="wq")
        wk_sb = consts.tile([128, KK, 128], bf16, tag="wk")
        wv_sb = consts.tile([128, KK, 128], bf16, tag="wv")
        wo_sb = consts.tile([128, HID], bf16, tag="wo")
        mask_sb = consts.tile([128, G + 6 * CH], bf16, tag="mask")
        ones_sb = consts.tile([1, 64], bf16, tag="ones")
        nc.sync.dma_start(wq_sb[:], wq_r)
        nc.sync.dma_start(wk_sb[:], wk_r)
        nc.sync.dma_start(wv_sb[:], wv_r)
        nc.sync.dma_start(wo_sb[:], wo_d[:])
        nc.sync.dma_start(mask_sb[:], mask_d[:])
        nc.vector.memset(ones_sb[:], 1.0)

        def finish_group(pvs, t0):
            # normalize (unnormalized PV x broadcast reciprocal) + out-projection
            rec = norm_pool.tile([1, 2 * G], bf16, tag="rec")
            with nc.allow_low_precision(reason="softmax denominator reciprocal in bf16"):
                nc.vector.reciprocal(rec[:], pvs[64:65, :])
            bcp = ps_mm.tile([128, G], f32, tag="mm")
            nc.tensor.matmul(bcp[0:64, :], ones_sb[:], rec[0:1, 0:G])
            nc.tensor.matmul(
                bcp[64:128, :], ones_sb[:], rec[0:1, G : 2 * G], tile_position=(0, 64)
            )
            at = attn_pool.tile([128, G], bf16, tag="attnT")
            nc.vector.tensor_tensor(at[0:64, :], pvs[0:64, 0:G], bcp[0:64, :], op=mult)
            nc.vector.tensor_tensor(
                at[64:128, :], pvs[0:64, G : 2 * G], bcp[64:128, :], op=mult
            )
            for tch in range(G // CH):
                for nn in range(2):
                    pso = ps_mm.tile([128, G], f32, tag="mm")
                    nc.tensor.matmul(
                        pso[:],
                        at[:, tch * CH : (tch + 1) * CH],
                        wo_sb[:, nn * G : (nn + 1) * G],
                    )
                    ob = osb_pool.tile([128, G], f32, tag="ob")
                    nc.vector.tensor_copy(ob[:], pso[:])
                    nc.sync.dma_start(
                        out_d[t0 + tch * CH : t0 + (tch + 1) * CH, nn * G : (nn + 1) * G],
                        ob[:],
                    )

        pending = None
        for _rep in range(reps):
            QTb, KTb, Vb = [], [], []
            for b in range(B):
                QTb.append(qkv_pool.tile([128, S], bf16, tag=f"QT{b}", name=f"QT{b}"))
                KTb.append(qkv_pool.tile([128, S], bf16, tag=f"KT{b}", name=f"KT{b}"))
                Vb.append(qkv_pool.tile([128, NKC, 2, 65], bf16, tag=f"V{b}", name=f"V{b}"))
                nc.gpsimd.memset(Vb[b][:, :, :, 64:65], 1.0)

            # interleave the two batches' query groups: (b0,g0),(b1,g0),(b0,g1),...
            # so small-g groups never drain the PE/ACT pipeline.
            for g, b in [(gg, bb) for gg in range(NGB) for bb in range(B)]:
                QT, KT, V = QTb[b], KTb[b], Vb[b]
                t0 = b * S + g * G

                # ---- phase 1: qkv projection for this token group ----
                xt = xt_pool.tile([128, KK, G], bf16, tag="xt")
                nc.sync.dma_start(xt[:], xT_r[:, :, t0 : t0 + G])
                for w_sb, dstT in ((wq_sb, QT), (wk_sb, KT)):
                    ps = ps_mm.tile([128, G], f32, tag="mm")
                    for kk in range(KK):
                        nc.tensor.matmul(
                            ps[:],
                            w_sb[:, kk, :],
                            xt[:, kk, :],
                            start=(kk == 0),
                            stop=(kk == KK - 1),
                        )
                    nc.vector.tensor_copy(dstT[:, g * G : (g + 1) * G], ps[:])
                for tch in range(G // CH):
                    ps = ps_mm.tile([128, G], f32, tag="mm")
                    for kk in range(KK):
                        nc.tensor.matmul(
                            ps[:, 0:CH],
                            xt[:, kk, tch * CH : (tch + 1) * CH],
                            wv_sb[:, kk, :],
                            start=(kk == 0),
                            stop=(kk == KK - 1),
                        )
                    kc = g * 4 + tch
                    nc.vector.tensor_copy(
                        V[:, kc, :, 0:64],
                        ps[:, 0:CH].rearrange("p (h d) -> p h d", h=2),
                    )

                # ---- phase 2: attention for query group g (keys 0..4g+3) ----
                nkc = 4 * g + 4  # causal key chunks for this group
                pv = [ps_pv.tile([65, G], f32, tag="pv", name=f"pv{h}") for h in range(2)]
                # packed mask offsets: jg=0..3 regions at [0:512],[512:896],[896:1152],[1152:1280]
                MOFF = [0, G, G + 3 * CH, G + 3 * CH + 2 * CH]
                for qd in range(nkc // 2):
                    # per-block trim: diagonal-band chunk jg only attends q >= jg*128,
                    # so its scores/exp/PV only cover N = 512 - jg*128 columns.
                    kcs, qoffs, Ns, offs = [], [], [], []
                    off = 0
                    for j in range(2):
                        kc = qd * 2 + j
                        jg = kc - (nkc - 4)
                        qoff = max(jg, 0) * CH
                        kcs.append(kc)
                        qoffs.append(qoff)
                        Ns.append(G - qoff)
                        offs.append(off)
                        off += G - qoff
                    sq = [ps_sq.tile([128, 2 * G], f32, tag="sq", name=f"sq{h}") for h in range(2)]
                    for j in range(2):
                        for h in range(2):
                            nc.tensor.matmul(
                                sq[h][:, offs[j] : offs[j] + Ns[j]],
                                KT[h * 64 : (h + 1) * 64, kcs[j] * CH : (kcs[j] + 1) * CH],
                                QT[h * 64 : (h + 1) * 64, g * G + qoffs[j] : (g + 1) * G],
                            )
                    ex = [exp_pool.tile([128, 2 * G], bf16, tag="exp", name=f"ex{h}") for h in range(2)]
                    for h in range(2):
                        # exp(scores / sqrt(D)); scale folded into the ACT affine
                        nc.scalar.activation(
                            ex[h][:, 0:off], sq[h][:, 0:off], Exp, scale=0.125
                        )
                    if qd >= nkc // 2 - 2:
                        seg = qd - (nkc // 2 - 2)  # 0 -> jg 0/1 region, 1 -> jg 2/3
                        moff = MOFF[2 * seg]
                        for h in range(2):
                            nc.vector.tensor_tensor(
                                ex[h][:, 0:off],
                                ex[h][:, 0:off],
                                mask_sb[:, moff : moff + off],
                                op=mult,
                            )
                    for j in range(2):
                        for h in range(2):
                            nc.tensor.matmul(
                                pv[h][:, qoffs[j] : G],
                                V[:, kcs[j], h, :],
                                ex[h][:, offs[j] : offs[j] + Ns[j]],
                                start=(kcs[j] == 0),
                                stop=(kcs[j] == nkc - 1),
                            )

                # free the pv psum banks immediately; normalize+outproj for this
                # group are emitted AFTER the next group's matmuls (1-group lag)
                # so the PE program never blocks on the DVE normalize chain.
                pvs = norm_pool.tile([65, 2 * G], f32, tag="pvs")
                nc.vector.tensor_copy(pvs[:, 0:G], pv[0][:])
                nc.vector.tensor_copy(pvs[:, G : 2 * G], pv[1][:])
                if pending is not None:
                    finish_group(*pending)
                pending = (pvs, t0)
        if pending is not None:
            finish_group(*pending)
    nc.compile()
    return nc


def _causal_mask():
    # Packed per-region triangular mask. Every diagonal-band chunk jg reduces to
    # the same pattern tri[k, qq] = (qq >= k) truncated to width 512-128*jg;
    # regions are packed back to back: widths 512, 384, 256, 128.
    tri = (np.arange(G)[None, :] >= np.arange(128)[:, None]).astype(np.float32)
    parts = [tri[:, 0 : G - jg * CH] for jg in range(4)]
    return np.concatenate(parts, axis=1).astype(ml_dtypes.bfloat16)


def get_nc(reps=1):
    key = f"nc{reps}"
    if key not in _CACHE:
        _CACHE[key] = _build_nc(reps)
    return _CACHE[key]


def make_in_maps(x, w_qkv, w_out):
    bf16 = ml_dtypes.bfloat16
    xf = np.asarray(x, dtype=np.float32).reshape(TT, HID)
    xT = np.ascontiguousarray(xf.T).astype(bf16)
    wqkv = np.asarray(w_qkv, dtype=np.float32)
    wout = np.asarray(w_out, dtype=np.float32)
    mask = _causal_mask()
    in_maps = []
    for c in range(8):
        c0 = 128 * c
        in_maps.append(
            {
                "xT": xT,
                "wq": np.ascontiguousarray(wqkv[:, c0 : c0 + 128]).astype(bf16),
                "wk": np.ascontiguousarray(wqkv[:, HID + c0 : HID + c0 + 128]).astype(bf16),
                "wv": np.ascontiguousarray(wqkv[:, 2 * HID + c0 : 2 * HID + c0 + 128]).astype(bf16),
                "wo": np.ascontiguousarray(wout[c0 : c0 + 128, :]).astype(bf16),
                "mask": mask,
            }
        )
    return in_maps


def kernel(x, w_qkv, w_out):
    from concourse.bass_utils import run_bass_kernel_spmd

    nc = get_nc()
    in_maps = make_in_maps(x, w_qkv, w_out)
    res = run_bass_kernel_spmd(nc, in_maps, core_ids=list(range(8)))
    acc = np.zeros((TT, HID), dtype=np.float32)
    for r in res.results:
        acc += r["out"]
    return acc.reshape(B, S, HID)

